# revision 1
# baseline (speedup 1.0000x reference)
import numpy as np
import jax
import jax.numpy as jnp
from jax import lax

# Nystrom attention, hardcoded problem shapes
H = 16        # num heads
DH = 64       # dim per head
M = 256       # num landmarks
ITERS = 6     # Moore-Penrose iterations
K = 13        # residual depthwise conv kernel size
B = 4
N = 4096
DIM = 1024
INNER = H * DH
HG = 2        # head groups (tensor-parallel over heads)
HPG = H // HG # heads per group
NCORES = 8    # B * HG

_SCALE = DH ** -0.5


def _device_fn(x_b, wq_g, wk_g, wv_g, w_conv_g, w_fc_g):
    # x_b [N,DIM]; wq/wk/wv [DIM, HPG*DH]; w_conv_g [HPG,1,K,1]; w_fc_g [HPG*DH, DIM]
    q = (x_b @ wq_g).reshape(N, HPG, DH).transpose(1, 0, 2) * _SCALE
    k = (x_b @ wk_g).reshape(N, HPG, DH).transpose(1, 0, 2)
    v = (x_b @ wv_g).reshape(N, HPG, DH).transpose(1, 0, 2)

    l = N // M
    q_l = q.reshape(HPG, M, l, DH).sum(axis=2) / l
    k_l = k.reshape(HPG, M, l, DH).sum(axis=2) / l

    sim1 = jnp.einsum('hid,hjd->hij', q, k_l)
    sim2 = jnp.einsum('hid,hjd->hij', q_l, k_l)
    sim3 = jnp.einsum('hid,hjd->hij', q_l, k)

    attn1 = jax.nn.softmax(sim1, axis=-1)
    attn2 = jax.nn.softmax(sim2, axis=-1)
    attn3 = jax.nn.softmax(sim3, axis=-1)

    # Moore-Penrose pseudo-inverse; the init normalizer is a GLOBAL max over
    # all (b, h) in the reference, so pmax across the 8 shards.
    a = attn2
    a_abs = jnp.abs(a)
    col = a_abs.sum(axis=-1)
    row = a_abs.sum(axis=-2)
    colmax = lax.pmax(jnp.max(col), axis_name='i')
    rowmax = lax.pmax(jnp.max(row), axis_name='i')
    z = jnp.swapaxes(a, -1, -2) / (colmax * rowmax)
    I = jnp.eye(M, dtype=a.dtype)
    for _ in range(ITERS):
        az = a @ z
        z = 0.25 * z @ (13 * I - az @ (15 * I - az @ (7 * I - az)))

    out = (attn1 @ z) @ (attn3 @ v)          # [HPG, N, DH]

    res = lax.conv_general_dilated(
        out[None], w_conv_g, window_strides=(1, 1),
        padding=((K // 2, K // 2), (0, 0)),
        feature_group_count=HPG,
        dimension_numbers=('NCHW', 'OIHW', 'NCHW'))[0]
    out = out + res

    out = out.transpose(1, 0, 2).reshape(N, HPG * DH)
    return out @ w_fc_g                       # [N, DIM] partial sum over head group


_pmapped = jax.pmap(_device_fn, axis_name='i')


def kernel(x, w_qkv, w_conv, w_fc, b_fc):
    x = np.asarray(x, dtype=np.float32)
    w_qkv = np.asarray(w_qkv, dtype=np.float32)
    w_conv = np.asarray(w_conv, dtype=np.float32)
    w_fc = np.asarray(w_fc, dtype=np.float32)
    b_fc = np.asarray(b_fc, dtype=np.float32)

    wq = w_qkv[:, :INNER].reshape(DIM, H, DH)
    wk = w_qkv[:, INNER:2 * INNER].reshape(DIM, H, DH)
    wv = w_qkv[:, 2 * INNER:].reshape(DIM, H, DH)
    w_fc_r = w_fc.reshape(H, DH, DIM)

    xs, wqs, wks, wvs, wcs, wfs = [], [], [], [], [], []
    for d in range(NCORES):
        b, g = divmod(d, HG)
        hs = slice(g * HPG, (g + 1) * HPG)
        xs.append(x[b])
        wqs.append(wq[:, hs, :].reshape(DIM, HPG * DH))
        wks.append(wk[:, hs, :].reshape(DIM, HPG * DH))
        wvs.append(wv[:, hs, :].reshape(DIM, HPG * DH))
        wcs.append(w_conv[hs])
        wfs.append(w_fc_r[hs].reshape(HPG * DH, DIM))

    out_parts = _pmapped(np.stack(xs), np.stack(wqs), np.stack(wks),
                         np.stack(wvs), np.stack(wcs), np.stack(wfs))
    out_parts = np.asarray(out_parts)                       # [8, N, DIM]
    out = out_parts.reshape(B, HG, N, DIM).sum(axis=1) + b_fc
    return out.astype(np.float32)
